# revision 36
# baseline (speedup 1.0000x reference)
"""Single-head causal self-attention on 8 Trainium2 NeuronCores (Bass/Tile).

Problem: x [1024, 256, 384], Wq/Wk/Wv [384, 64] ->
  q,k,v = x@W;  wei = softmax(mask(q k^T / sqrt(384)));  out = wei @ v
Output: [1024, 256, 64] fp32.

Strategy (data-parallel over batch, 128 batches = 64 pairs per core):
  - Layout-by-stationarity: every tensor is produced in exactly the layout
    its consumer needs, so there are NO transposes anywhere.
      q^T,k^T [h, t]  = W-stationary matmul   (lhsT = [Wq|Wk] chunk)
      v       [s, h]  = x-stationary matmul   (lhsT = x^T chunk)
      P       [s, t]  = k-stationary matmul   (lhsT = k^T s-half) + exp
      out     [t, h]  = P-stationary matmul   (lhsT = P block, rhs = [v|1])
  - The ones column appended to v makes the out-MM emit the softmax
    denominator per OUTPUT PARTITION (= token), so normalization is one
    cheap reciprocal [128, 4, 1] + one broadcast tensor_mul per pair.
  - Causal structure at 128-block granularity: tokens t<128 never touch
    s-half1, so wei needs only 3 of 4 blocks (A=s0xt0 diag, B=s0xt1 full,
    C=s1xt1 diag); exp covers [128, 384]; mask [triu|triu] on A,C only.
  - bf16 operand pipeline (fp32 PSUM accumulation): 1 elem/cycle PE
    streaming (fp32r is ~2 cyc/elem), FWL fast weight loads, half HBM.
  - Software-pipelined emission: the pair loop is skewed into 6 stages
    (DMA / projections / PSUM->SBUF copies / wei+exp+mask / out-MMs /
    normalize+store) so each engine sees work from several pairs back to
    back. The projection stage alone is 15 dense matmuls, which keeps the
    PE HAM clock-gate warm (K=8/8).
"""

import os
from contextlib import ExitStack

import ml_dtypes
import numpy as np

import concourse.bass as bass
import concourse.bacc as bacc
import concourse.tile as tile
from concourse import mybir
from concourse.bass import broadcast_tensor_aps
from concourse.bass_utils import run_bass_kernel_spmd

N_CORES = 8
B = 1024
T = 256
C = 384
H = 64
BPC = B // N_CORES  # 128 batches per core
NPAIR = BPC // 2  # 64 pairs per core
NCHUNK = C // 128  # 3
SCALE = float(C) ** -0.5
GW = H + 2  # [v | 1 | pad] group width (even free dim, 8B-aligned groups)

F32 = mybir.dt.float32
BF16 = mybir.dt.bfloat16
BF = ml_dtypes.bfloat16


def build_nc(npair: int = NPAIR):
    nc = bacc.Bacc(
        "TRN2", target_bir_lowering=False, debug=False, num_devices=N_CORES
    )

    # xT[pr, p, c, 256*b2 + t] = x[2*pr + b2, t, 128*c + p]
    xT = nc.dram_tensor("xT", [npair, 128, NCHUNK, 2 * T], BF16, kind="ExternalInput").ap()
    wqk = nc.dram_tensor("wqk", [128, NCHUNK, 128], BF16, kind="ExternalInput").ap()
    wv = nc.dram_tensor("wv", [128, NCHUNK, H], BF16, kind="ExternalInput").ap()
    # [triu | triu] for the A and C (diagonal) block columns of P
    mask = nc.dram_tensor("mask", [128, 256], BF16, kind="ExternalInput").ap()
    # outH[pr, p, 64*g + h] = out[2*pr + g//2, 128*(g%2) + p, h]
    outH = nc.dram_tensor("outH", [npair, 128, 4 * H], F32, kind="ExternalOutput").ap()

    with ExitStack() as ctx:
        tc = ctx.enter_context(tile.TileContext(nc))

        const = ctx.enter_context(tc.tile_pool(name="const", bufs=1))
        wqk_sb = const.tile([128, NCHUNK, 128], BF16, tag="wqk")
        nc.sync.dma_start(wqk_sb[:], wqk)
        wv_sb = const.tile([128, NCHUNK, H], BF16, tag="wv")
        nc.sync.dma_start(wv_sb[:], wv)
        mask_sb = const.tile([128, 256], BF16, tag="mask")
        nc.sync.dma_start(mask_sb[:], mask)

        # Persistent [v | 1 | pad] rhs tiles: 4 groups (b0s0, b0s1, b1s0,
        # b1s1), each [128, 66] with ones at cols 64:66.
        NSLOT = 4
        vaug = []
        for i in range(NSLOT):
            v_t = const.tile([128, 4, GW], BF16, tag=f"vaug{i}")
            nc.gpsimd.memset(v_t[:, :, H : H + 2], 1.0)
            vaug.append(v_t)

        # q^T/k^T slots padded to 128 partitions with persistent zero rows:
        # the wei matmuls then contract over K=128, which qualifies their
        # weight loads for FWL (fast weight load needs 128 weight rows).
        qslots, kslots = [], []
        for i in range(NSLOT):
            q_t = const.tile([128, 512], BF16, tag=f"qs{i}")
            nc.gpsimd.memset(q_t[H:128, :], 0.0)
            qslots.append(q_t)
            k_t = const.tile([128, 512], BF16, tag=f"ks{i}")
            nc.gpsimd.memset(k_t[H:128, :], 0.0)
            kslots.append(k_t)

        xt_pool = ctx.enter_context(tc.tile_pool(name="xt", bufs=6))
        p_pool = ctx.enter_context(tc.tile_pool(name="pp", bufs=3))
        o_pool = ctx.enter_context(tc.tile_pool(name="op", bufs=6))
        r_pool = ctx.enter_context(tc.tile_pool(name="rp", bufs=6))

        psqk_pool = ctx.enter_context(tc.tile_pool(name="psqk", bufs=2, space="PSUM"))
        psv_pool = ctx.enter_context(tc.tile_pool(name="psv", bufs=2, space="PSUM"))
        psc_pool = ctx.enter_context(tc.tile_pool(name="psc", bufs=2, space="PSUM"))
        pso_pool = ctx.enter_context(tc.tile_pool(name="pso", bufs=2, space="PSUM"))

        st = [dict() for _ in range(npair)]

        def s0_dma(pr):
            xt = xt_pool.tile([128, NCHUNK, 2 * T], BF16, tag="xt")
            nc.sync.dma_start(xt[:], xT[pr])
            st[pr]["xt"] = xt

        def s1_proj(pr):
            xt = st[pr]["xt"]
            # q^T|k^T for the pair: [128 qk-dims, 512 tokens]
            ps_qk = psqk_pool.tile([128, 512], F32, tag="psqk")
            for c in range(NCHUNK):
                nc.tensor.matmul(
                    ps_qk[:],
                    lhsT=wqk_sb[:, c, :],
                    rhs=xt[:, c, :],
                    start=(c == 0),
                    stop=(c == NCHUNK - 1),
                )
            # v [s, h] via x-stationary matmuls: group g = 2*b2 + s
            ps_v = psv_pool.tile([128, 512], F32, tag="psv")
            for g in range(4):
                b2, s = divmod(g, 2)
                t0 = b2 * T + s * 128
                for c in range(NCHUNK):
                    nc.tensor.matmul(
                        ps_v[:, g * H : (g + 1) * H],
                        lhsT=xt[:, c, t0 : t0 + 128],
                        rhs=wv_sb[:, c, :],
                        start=(c == 0),
                        stop=(c == NCHUNK - 1),
                    )
            st[pr]["ps_qk"] = ps_qk
            st[pr]["ps_v"] = ps_v

        def s2_copies(pr):
            ps_qk = st[pr]["ps_qk"]
            ps_v = st[pr]["ps_v"]
            q_sb = qslots[pr % NSLOT]
            nc.scalar.copy(q_sb[0:H, :], ps_qk[0:H, :])
            k_sb = kslots[pr % NSLOT]
            nc.vector.tensor_copy(k_sb[0:H, :], ps_qk[H:128, :])
            v_sb = vaug[pr % NSLOT]
            nc.vector.tensor_copy(
                v_sb[:, :, 0:H],
                ps_v[:, 0:256].rearrange("p (g h) -> p g h", g=4),
            )
            st[pr]["q"] = q_sb
            st[pr]["k"] = k_sb
            st[pr]["v"] = v_sb

        def s3_mm(pr):
            # wei matmuls FIRST in the tick's PE queue so the exps unblock
            # ~0.4us in instead of after the 15 projection matmuls.
            q_sb, k_sb = st[pr]["q"], st[pr]["k"]
            cs = []
            for b2 in range(2):
                toff = b2 * T
                # wei blocks [s, t]: A|B = (s0, t0:256) one N=256 matmul,
                # C = (s1, t1) one N=128 matmul. (t<128 never sees s1.)
                ps_c = psc_pool.tile([128, 512], F32, tag="psc")
                nc.tensor.matmul(
                    ps_c[:, 0:256],
                    lhsT=k_sb[:, toff : toff + 128],
                    rhs=q_sb[:, toff : toff + T],
                    start=True,
                    stop=True,
                )
                nc.tensor.matmul(
                    ps_c[:, 256:384],
                    lhsT=k_sb[:, toff + 128 : toff + 256],
                    rhs=q_sb[:, toff + 128 : toff + 256],
                    start=True,
                    stop=True,
                )
                cs.append(ps_c)
            st[pr]["c"] = cs

        def s3_act(pr):
            # exps BEFORE the q-copy in the ACT queue (deps: same-tick weiT,
            # which now leads the PE queue).
            cs = st[pr]["c"]
            p_sb = p_pool.tile([128, 2, 384], BF16, tag="p")
            for b2 in range(2):
                nc.scalar.activation(
                    p_sb[:, b2, :], cs[b2][:, 0:384],
                    mybir.ActivationFunctionType.Exp, scale=SCALE,
                )
            st[pr]["p"] = p_sb

        def s3_mask(pr):
            # causal mask on the diagonal blocks (A at 0:128, C at 256:384).
            # b0's mask is ready early -> slow-but-idle GpSimd; b1's mask
            # trails the second exp -> fast DVE so it lands within the tick.
            p_sb = st[pr]["p"]
            mv = mask_sb[:].rearrange("p (b x) -> p b x", x=128)
            pv0 = p_sb[:, 0, :].rearrange("p (b x) -> p b x", x=128)[:, 0:3:2, :]
            nc.gpsimd.tensor_mul(pv0, pv0, mv)
            pv1 = p_sb[:, 1, :].rearrange("p (b x) -> p b x", x=128)[:, 0:3:2, :]
            nc.vector.tensor_mul(pv1, pv1, mv)

        def s4_out(pr):
            v_sb = st[pr]["v"]
            p_sb = st[pr]["p"]
            ps_o = pso_pool.tile([128, 512], F32, tag="pso")
            for b2 in range(2):
                g0 = 2 * b2
                # out[t, 0:64] + denom[t] (col 64) via P-stationary matmuls
                nc.tensor.matmul(
                    ps_o[:, g0 * GW : g0 * GW + GW],
                    lhsT=p_sb[:, b2, 0:128],
                    rhs=v_sb[:, g0, :],
                    start=True,
                    stop=True,
                )
                nc.tensor.matmul(
                    ps_o[:, (g0 + 1) * GW : (g0 + 2) * GW],
                    lhsT=p_sb[:, b2, 128:256],
                    rhs=v_sb[:, g0, :],
                    start=True,
                    stop=False,
                )
                nc.tensor.matmul(
                    ps_o[:, (g0 + 1) * GW : (g0 + 2) * GW],
                    lhsT=p_sb[:, b2, 256:384],
                    rhs=v_sb[:, g0 + 1, :],
                    start=False,
                    stop=True,
                )
            st[pr]["ps_o"] = ps_o

        def s5_norm(pr):
            ps_o = st[pr]["ps_o"]
            # normalize: out[t, h] / denom[t] for all 4 groups at once
            og = ps_o[:, 0 : 4 * GW].rearrange("p (g c) -> p g c", g=4)
            rs = r_pool.tile([128, 4], F32, tag="rs")
            rsv = rs[:].rearrange("p (g c) -> p g c", c=1)
            nc.vector.reciprocal(rsv, og[:, :, H : H + 1])
            out_sb = o_pool.tile([128, 4, H], F32, tag="out")
            in0, in1 = broadcast_tensor_aps(og[:, :, 0:H], rsv)
            nc.vector.tensor_mul(out_sb[:], in0, in1)
            st[pr]["out"] = out_sb

        def s6_store(pr):
            # A tick after norm, so this never stalls the Sync FIFO and the
            # next tick's input DMA issues immediately behind it.
            nc.sync.dma_start(outH[pr], st[pr]["out"][:])
            st[pr].clear()

        # Within-tick emission order controls each engine's FIFO order:
        # PE gets weiT -> out-MMs -> projections; ACT gets exps -> q-copy;
        # DVE gets recip/norm -> k/v copies -> trailing mask.
        stages = [
            (3, s3_mm),
            (4, s4_out),
            (5, s5_norm),
            (3, s3_act),
            (0, s0_dma),
            (1, s1_proj),
            (2, s2_copies),
            (3, s3_mask),
            (6, s6_store),
        ]
        for t in range(npair + 6):
            for off, stage in stages:
                pr = t - off
                if 0 <= pr < npair:
                    stage(pr)

    nc.finalize()
    return nc


def _host_inputs(x, Wq, Wk, Wv):
    B_, T_, C_ = x.shape
    assert (B_, T_, C_) == (B, T, C), (B_, T_, C_)
    # xh[pr, p, c, 256*b2 + t] = x[2*pr + b2, t, 128*c + p]
    xh = np.ascontiguousarray(
        x.reshape(B // 2, 2, T, NCHUNK, 128).transpose(0, 4, 3, 1, 2)
        .reshape(B // 2, 128, NCHUNK, 2 * T)
        .astype(BF)
    )
    wqk_h = np.ascontiguousarray(
        np.concatenate([Wq, Wk], axis=1).reshape(NCHUNK, 128, 128).transpose(1, 0, 2)
        .astype(BF)
    )
    wv_h = np.ascontiguousarray(
        Wv.reshape(NCHUNK, 128, H).transpose(1, 0, 2).astype(BF)
    )
    triu = np.triu(np.ones((128, 128), dtype=BF))
    mask_h = np.ascontiguousarray(np.concatenate([triu, triu], axis=1))
    return xh, wqk_h, wv_h, mask_h


def _make_in_maps(xh, wqk_h, wv_h, mask_h):
    return [
        {
            "xT": xh[i * NPAIR : (i + 1) * NPAIR],
            "wqk": wqk_h,
            "wv": wv_h,
            "mask": mask_h,
        }
        for i in range(N_CORES)
    ]


def _assemble(results):
    # outH per core: [NPAIR, 128, 256]; groups g = 2*b2 + thalf
    outH = np.concatenate([results[i]["outH"] for i in range(N_CORES)], axis=0)
    out = (
        outH.reshape(B // 2, 128, 2, 2, H)
        .transpose(0, 2, 3, 1, 4)
        .reshape(B, T, H)
    )
    return np.ascontiguousarray(out)


def kernel(x, Wq, Wk, Wv):
    x = np.asarray(x, dtype=np.float32)
    Wq = np.asarray(Wq, dtype=np.float32)
    Wk = np.asarray(Wk, dtype=np.float32)
    Wv = np.asarray(Wv, dtype=np.float32)

    xh, wqk_h, wv_h, mask_h = _host_inputs(x, Wq, Wk, Wv)
    nc = build_nc(NPAIR)
    in_maps = _make_in_maps(xh, wqk_h, wv_h, mask_h)
    res = run_bass_kernel_spmd(nc, in_maps, list(range(N_CORES)))
    return _assemble(res.results)


# revision 37
# speedup vs baseline: 1.0118x; 1.0118x over previous
"""Single-head causal self-attention on 8 Trainium2 NeuronCores (Bass/Tile).

Problem: x [1024, 256, 384], Wq/Wk/Wv [384, 64] ->
  q,k,v = x@W;  wei = softmax(mask(q k^T / sqrt(384)));  out = wei @ v
Output: [1024, 256, 64] fp32.

Strategy (data-parallel over batch, 128 batches = 64 pairs per core):
  - Layout-by-stationarity: every tensor is produced in exactly the layout
    its consumer needs, so there are NO transposes anywhere.
      q^T,k^T [h, t]  = W-stationary matmul   (lhsT = [Wq|Wk] chunk)
      v       [s, h]  = x-stationary matmul   (lhsT = x^T chunk)
      P       [s, t]  = k-stationary matmul   (lhsT = k^T s-half) + exp
      out     [t, h]  = P-stationary matmul   (lhsT = P block, rhs = [v|1])
  - The ones column appended to v makes the out-MM emit the softmax
    denominator per OUTPUT PARTITION (= token), so normalization is one
    cheap reciprocal [128, 4, 1] + one broadcast tensor_mul per pair.
  - Causal structure at 128-block granularity: tokens t<128 never touch
    s-half1, so wei needs only 3 of 4 blocks (A=s0xt0 diag, B=s0xt1 full,
    C=s1xt1 diag); exp covers [128, 384]; mask [triu|triu] on A,C only.
  - bf16 operand pipeline (fp32 PSUM accumulation): 1 elem/cycle PE
    streaming (fp32r is ~2 cyc/elem), FWL fast weight loads, half HBM.
  - Software-pipelined emission: the pair loop is skewed into 6 stages
    (DMA / projections / PSUM->SBUF copies / wei+exp+mask / out-MMs /
    normalize+store) so each engine sees work from several pairs back to
    back. The projection stage alone is 15 dense matmuls, which keeps the
    PE HAM clock-gate warm (K=8/8).
"""

import os
from contextlib import ExitStack

import ml_dtypes
import numpy as np

import concourse.bass as bass
import concourse.bacc as bacc
import concourse.tile as tile
from concourse import mybir
from concourse.bass import broadcast_tensor_aps
from concourse.bass_utils import run_bass_kernel_spmd

N_CORES = 8
B = 1024
T = 256
C = 384
H = 64
BPC = B // N_CORES  # 128 batches per core
NPAIR = BPC // 2  # 64 pairs per core
NCHUNK = C // 128  # 3
SCALE = float(C) ** -0.5
GW = H + 2  # [v | 1 | pad] group width (even free dim, 8B-aligned groups)

F32 = mybir.dt.float32
BF16 = mybir.dt.bfloat16
BF = ml_dtypes.bfloat16


def build_nc(npair: int = NPAIR):
    nc = bacc.Bacc(
        "TRN2", target_bir_lowering=False, debug=False, num_devices=N_CORES
    )

    # xT[pr, p, c, 256*b2 + t] = x[2*pr + b2, t, 128*c + p]
    xT = nc.dram_tensor("xT", [npair, 128, NCHUNK, 2 * T], BF16, kind="ExternalInput").ap()
    wqk = nc.dram_tensor("wqk", [128, NCHUNK, 128], BF16, kind="ExternalInput").ap()
    wv = nc.dram_tensor("wv", [128, NCHUNK, H], BF16, kind="ExternalInput").ap()
    # [triu | triu] for the A and C (diagonal) block columns of P
    mask = nc.dram_tensor("mask", [128, 256], BF16, kind="ExternalInput").ap()
    # outH[pr, p, 64*g + h] = out[2*pr + g//2, 128*(g%2) + p, h]
    outH = nc.dram_tensor("outH", [npair, 128, 4 * H], F32, kind="ExternalOutput").ap()

    with ExitStack() as ctx:
        tc = ctx.enter_context(tile.TileContext(nc))

        const = ctx.enter_context(tc.tile_pool(name="const", bufs=1))
        wqk_sb = const.tile([128, NCHUNK, 128], BF16, tag="wqk")
        nc.sync.dma_start(wqk_sb[:], wqk)
        wv_sb = const.tile([128, NCHUNK, H], BF16, tag="wv")
        nc.sync.dma_start(wv_sb[:], wv)
        mask_sb = const.tile([128, 256], BF16, tag="mask")
        nc.sync.dma_start(mask_sb[:], mask)

        # Persistent [v | 1 | pad] rhs tiles: 4 groups (b0s0, b0s1, b1s0,
        # b1s1), each [128, 66] with ones at cols 64:66.
        NSLOT = 4
        vaug = []
        for i in range(NSLOT):
            v_t = const.tile([128, 4, GW], BF16, tag=f"vaug{i}")
            nc.gpsimd.memset(v_t[:, :, H : H + 2], 1.0)
            vaug.append(v_t)

        # q^T/k^T slots padded to 128 partitions with persistent zero rows:
        # the wei matmuls then contract over K=128, which qualifies their
        # weight loads for FWL (fast weight load needs 128 weight rows).
        qslots, kslots = [], []
        for i in range(NSLOT):
            q_t = const.tile([128, 512], BF16, tag=f"qs{i}")
            nc.gpsimd.memset(q_t[H:128, :], 0.0)
            qslots.append(q_t)
            k_t = const.tile([128, 512], BF16, tag=f"ks{i}")
            nc.gpsimd.memset(k_t[H:128, :], 0.0)
            kslots.append(k_t)

        xt_pool = ctx.enter_context(tc.tile_pool(name="xt", bufs=6))
        p_pool = ctx.enter_context(tc.tile_pool(name="pp", bufs=3))
        o_pool = ctx.enter_context(tc.tile_pool(name="op", bufs=6))
        r_pool = ctx.enter_context(tc.tile_pool(name="rp", bufs=6))

        psqk_pool = ctx.enter_context(tc.tile_pool(name="psqk", bufs=2, space="PSUM"))
        psv_pool = ctx.enter_context(tc.tile_pool(name="psv", bufs=2, space="PSUM"))
        psc_pool = ctx.enter_context(tc.tile_pool(name="psc", bufs=2, space="PSUM"))
        pso_pool = ctx.enter_context(tc.tile_pool(name="pso", bufs=2, space="PSUM"))

        st = [dict() for _ in range(npair)]

        def s0_dma(pr):
            xt = xt_pool.tile([128, NCHUNK, 2 * T], BF16, tag="xt")
            nc.sync.dma_start(xt[:], xT[pr])
            st[pr]["xt"] = xt

        def s1_proj(pr):
            xt = st[pr]["xt"]
            # q^T|k^T for the pair: [128 qk-dims, 512 tokens]
            ps_qk = psqk_pool.tile([128, 512], F32, tag="psqk")
            for c in range(NCHUNK):
                nc.tensor.matmul(
                    ps_qk[:],
                    lhsT=wqk_sb[:, c, :],
                    rhs=xt[:, c, :],
                    start=(c == 0),
                    stop=(c == NCHUNK - 1),
                )
            # v [s, h] via x-stationary matmuls: group g = 2*b2 + s
            ps_v = psv_pool.tile([128, 512], F32, tag="psv")
            for g in range(4):
                b2, s = divmod(g, 2)
                t0 = b2 * T + s * 128
                for c in range(NCHUNK):
                    nc.tensor.matmul(
                        ps_v[:, g * H : (g + 1) * H],
                        lhsT=xt[:, c, t0 : t0 + 128],
                        rhs=wv_sb[:, c, :],
                        start=(c == 0),
                        stop=(c == NCHUNK - 1),
                    )
            st[pr]["ps_qk"] = ps_qk
            st[pr]["ps_v"] = ps_v

        def s2_copies(pr):
            ps_qk = st[pr]["ps_qk"]
            ps_v = st[pr]["ps_v"]
            q_sb = qslots[pr % NSLOT]
            nc.scalar.copy(q_sb[0:H, :], ps_qk[0:H, :])
            k_sb = kslots[pr % NSLOT]
            nc.vector.tensor_copy(k_sb[0:H, :], ps_qk[H:128, :])
            v_sb = vaug[pr % NSLOT]
            nc.vector.tensor_copy(
                v_sb[:, :, 0:H],
                ps_v[:, 0:256].rearrange("p (g h) -> p g h", g=4),
            )
            st[pr]["q"] = q_sb
            st[pr]["k"] = k_sb
            st[pr]["v"] = v_sb

        def s3_mm(pr):
            # wei matmuls FIRST in the tick's PE queue so the exps unblock
            # ~0.4us in instead of after the 15 projection matmuls.
            q_sb, k_sb = st[pr]["q"], st[pr]["k"]
            cs = []
            for b2 in range(2):
                toff = b2 * T
                # wei blocks [s, t]: A|B = (s0, t0:256) one N=256 matmul,
                # C = (s1, t1) one N=128 matmul. (t<128 never sees s1.)
                ps_c = psc_pool.tile([128, 512], F32, tag="psc")
                nc.tensor.matmul(
                    ps_c[:, 0:256],
                    lhsT=k_sb[:, toff : toff + 128],
                    rhs=q_sb[:, toff : toff + T],
                    start=True,
                    stop=True,
                )
                nc.tensor.matmul(
                    ps_c[:, 256:384],
                    lhsT=k_sb[:, toff + 128 : toff + 256],
                    rhs=q_sb[:, toff + 128 : toff + 256],
                    start=True,
                    stop=True,
                )
                cs.append(ps_c)
            st[pr]["c"] = cs

        def s3_act(pr):
            # exps BEFORE the q-copy in the ACT queue (deps: same-tick weiT,
            # which now leads the PE queue).
            cs = st[pr]["c"]
            p_sb = p_pool.tile([128, 2, 384], BF16, tag="p")
            for b2 in range(2):
                nc.scalar.activation(
                    p_sb[:, b2, :], cs[b2][:, 0:384],
                    mybir.ActivationFunctionType.Exp, scale=SCALE,
                )
            st[pr]["p"] = p_sb

        def s3_mask(pr):
            # causal mask on the diagonal blocks (A at 0:128, C at 256:384).
            # b0's mask is ready early -> slow-but-idle GpSimd; b1's mask
            # trails the second exp -> fast DVE so it lands within the tick.
            p_sb = st[pr]["p"]
            mv = mask_sb[:].rearrange("p (b x) -> p b x", x=128)
            pv0 = p_sb[:, 0, :].rearrange("p (b x) -> p b x", x=128)[:, 0:3:2, :]
            nc.gpsimd.tensor_mul(pv0, pv0, mv)
            pv1 = p_sb[:, 1, :].rearrange("p (b x) -> p b x", x=128)[:, 0:3:2, :]
            nc.vector.tensor_mul(pv1, pv1, mv)

        def s4_out(pr):
            v_sb = st[pr]["v"]
            p_sb = st[pr]["p"]
            ps_o = pso_pool.tile([128, 512], F32, tag="pso")
            for b2 in range(2):
                g0 = 2 * b2
                # out[t, 0:64] + denom[t] (col 64) via P-stationary matmuls
                nc.tensor.matmul(
                    ps_o[:, g0 * GW : g0 * GW + GW],
                    lhsT=p_sb[:, b2, 0:128],
                    rhs=v_sb[:, g0, :],
                    start=True,
                    stop=True,
                )
                nc.tensor.matmul(
                    ps_o[:, (g0 + 1) * GW : (g0 + 2) * GW],
                    lhsT=p_sb[:, b2, 128:256],
                    rhs=v_sb[:, g0, :],
                    start=True,
                    stop=False,
                )
                nc.tensor.matmul(
                    ps_o[:, (g0 + 1) * GW : (g0 + 2) * GW],
                    lhsT=p_sb[:, b2, 256:384],
                    rhs=v_sb[:, g0 + 1, :],
                    start=False,
                    stop=True,
                )
            st[pr]["ps_o"] = ps_o

        def s5_norm(pr):
            ps_o = st[pr]["ps_o"]
            # normalize: out[t, h] / denom[t] for all 4 groups at once
            og = ps_o[:, 0 : 4 * GW].rearrange("p (g c) -> p g c", g=4)
            rs = r_pool.tile([128, 4], F32, tag="rs")
            rsv = rs[:].rearrange("p (g c) -> p g c", c=1)
            nc.vector.reciprocal(rsv, og[:, :, H : H + 1])
            out_sb = o_pool.tile([128, 4, H], F32, tag="out")
            in0, in1 = broadcast_tensor_aps(og[:, :, 0:H], rsv)
            nc.vector.tensor_mul(out_sb[:], in0, in1)
            st[pr]["out"] = out_sb

        def s6_store(pr):
            # A tick after norm, so this never stalls the Sync FIFO and the
            # next tick's input DMA issues immediately behind it.
            nc.sync.dma_start(outH[pr], st[pr]["out"][:])
            st[pr].clear()

        stages = [
            (0, s0_dma),
            (1, s1_proj),
            (2, s2_copies),
            (3, s3_mm),
            (3, s3_act),
            (3, s3_mask),
            (4, s4_out),
            (5, s5_norm),
            (6, s6_store),
        ]
        for t in range(npair + 6):
            for off, stage in stages:
                pr = t - off
                if 0 <= pr < npair:
                    stage(pr)

    nc.finalize()
    return nc


def _host_inputs(x, Wq, Wk, Wv):
    B_, T_, C_ = x.shape
    assert (B_, T_, C_) == (B, T, C), (B_, T_, C_)
    # xh[pr, p, c, 256*b2 + t] = x[2*pr + b2, t, 128*c + p]
    xh = np.ascontiguousarray(
        x.reshape(B // 2, 2, T, NCHUNK, 128).transpose(0, 4, 3, 1, 2)
        .reshape(B // 2, 128, NCHUNK, 2 * T)
        .astype(BF)
    )
    wqk_h = np.ascontiguousarray(
        np.concatenate([Wq, Wk], axis=1).reshape(NCHUNK, 128, 128).transpose(1, 0, 2)
        .astype(BF)
    )
    wv_h = np.ascontiguousarray(
        Wv.reshape(NCHUNK, 128, H).transpose(1, 0, 2).astype(BF)
    )
    triu = np.triu(np.ones((128, 128), dtype=BF))
    mask_h = np.ascontiguousarray(np.concatenate([triu, triu], axis=1))
    return xh, wqk_h, wv_h, mask_h


def _make_in_maps(xh, wqk_h, wv_h, mask_h):
    return [
        {
            "xT": xh[i * NPAIR : (i + 1) * NPAIR],
            "wqk": wqk_h,
            "wv": wv_h,
            "mask": mask_h,
        }
        for i in range(N_CORES)
    ]


def _assemble(results):
    # outH per core: [NPAIR, 128, 256]; groups g = 2*b2 + thalf
    outH = np.concatenate([results[i]["outH"] for i in range(N_CORES)], axis=0)
    out = (
        outH.reshape(B // 2, 128, 2, 2, H)
        .transpose(0, 2, 3, 1, 4)
        .reshape(B, T, H)
    )
    return np.ascontiguousarray(out)


def kernel(x, Wq, Wk, Wv):
    x = np.asarray(x, dtype=np.float32)
    Wq = np.asarray(Wq, dtype=np.float32)
    Wk = np.asarray(Wk, dtype=np.float32)
    Wv = np.asarray(Wv, dtype=np.float32)

    xh, wqk_h, wv_h, mask_h = _host_inputs(x, Wq, Wk, Wv)
    nc = build_nc(NPAIR)
    in_maps = _make_in_maps(xh, wqk_h, wv_h, mask_h)
    res = run_bass_kernel_spmd(nc, in_maps, list(range(N_CORES)))
    return _assemble(res.results)


# revision 39
# speedup vs baseline: 1.1381x; 1.1248x over previous
"""Single-head causal self-attention on 8 Trainium2 NeuronCores (Bass/Tile).

Problem: x [1024, 256, 384], Wq/Wk/Wv [384, 64] ->
  q,k,v = x@W;  wei = softmax(mask(q k^T / sqrt(384)));  out = wei @ v
Output: [1024, 256, 64] fp32.

Strategy (data-parallel over batch, 128 batches = 64 pairs per core):
  - Layout-by-stationarity: every tensor is produced in exactly the layout
    its consumer needs, so there are NO transposes anywhere.
      q^T,k^T [h, t]  = W-stationary matmul   (lhsT = [Wq|Wk] chunk)
      v       [s, h]  = x-stationary matmul   (lhsT = x^T chunk)
      P       [s, t]  = k-stationary matmul   (lhsT = k^T s-half) + exp
      out     [t, h]  = P-stationary matmul   (lhsT = P block, rhs = [v|1])
  - The ones column appended to v makes the out-MM emit the softmax
    denominator per OUTPUT PARTITION (= token), so normalization is one
    cheap reciprocal [128, 4, 1] + one broadcast tensor_mul per pair.
  - Causal structure at 128-block granularity: tokens t<128 never touch
    s-half1, so wei needs only 3 of 4 blocks (A=s0xt0 diag, B=s0xt1 full,
    C=s1xt1 diag); exp covers [128, 384]; mask [triu|triu] on A,C only.
  - bf16 operand pipeline (fp32 PSUM accumulation): 1 elem/cycle PE
    streaming (fp32r is ~2 cyc/elem), FWL fast weight loads, half HBM.
  - Software-pipelined emission: the pair loop is skewed into 6 stages
    (DMA / projections / PSUM->SBUF copies / wei+exp+mask / out-MMs /
    normalize+store) so each engine sees work from several pairs back to
    back. The projection stage alone is 15 dense matmuls, which keeps the
    PE HAM clock-gate warm (K=8/8).
"""

import os
from contextlib import ExitStack

import ml_dtypes
import numpy as np

import concourse.bass as bass
import concourse.bacc as bacc
import concourse.tile as tile
from concourse import mybir
from concourse.bass import broadcast_tensor_aps
from concourse.bass_utils import run_bass_kernel_spmd

N_CORES = 8
B = 1024
T = 256
C = 384
H = 64
BPC = B // N_CORES  # 128 batches per core
NPAIR = BPC // 2  # 64 pairs per core
NCHUNK = C // 128  # 3
SCALE = float(C) ** -0.5
GW = H + 2  # [v | 1 | pad] group width (even free dim, 8B-aligned groups)

F32 = mybir.dt.float32
BF16 = mybir.dt.bfloat16
BF = ml_dtypes.bfloat16


def build_nc(npair: int = NPAIR):
    nc = bacc.Bacc(
        "TRN2", target_bir_lowering=False, debug=False, num_devices=N_CORES
    )

    # xT[pr, p, c, 256*b2 + t] = x[2*pr + b2, t, 128*c + p]
    xT = nc.dram_tensor("xT", [npair, 128, NCHUNK, 2 * T], BF16, kind="ExternalInput").ap()
    wqk = nc.dram_tensor("wqk", [128, NCHUNK, 128], BF16, kind="ExternalInput").ap()
    wv = nc.dram_tensor("wv", [128, NCHUNK, H], BF16, kind="ExternalInput").ap()
    # [triu | triu] for the A and C (diagonal) block columns of P
    mask = nc.dram_tensor("mask", [128, 256], BF16, kind="ExternalInput").ap()
    # outH[pr, p, 64*g + h] = out[2*pr + g//2, 128*(g%2) + p, h]
    outH = nc.dram_tensor("outH", [npair, 128, 4 * H], F32, kind="ExternalOutput").ap()

    with ExitStack() as ctx:
        tc = ctx.enter_context(tile.TileContext(nc))

        const = ctx.enter_context(tc.tile_pool(name="const", bufs=1))
        wqk_sb = const.tile([128, NCHUNK, 128], BF16, tag="wqk")
        nc.sync.dma_start(wqk_sb[:], wqk)
        wv_sb = const.tile([128, NCHUNK, H], BF16, tag="wv")
        nc.sync.dma_start(wv_sb[:], wv)
        mask_sb = const.tile([128, 256], BF16, tag="mask")
        nc.sync.dma_start(mask_sb[:], mask)

        # Persistent [v | 1 | pad] rhs tiles: 4 groups (b0s0, b0s1, b1s0,
        # b1s1), each [128, 66] with ones at cols 64:66.
        NSLOT = 4
        vaug = []
        for i in range(NSLOT):
            v_t = const.tile([128, 4, GW], BF16, tag=f"vaug{i}")
            nc.gpsimd.memset(v_t[:, :, H : H + 2], 1.0)
            vaug.append(v_t)

        # q^T/k^T slots padded to 128 partitions with persistent zero rows:
        # the wei matmuls then contract over K=128, which qualifies their
        # weight loads for FWL (fast weight load needs 128 weight rows).
        qslots, kslots = [], []
        for i in range(NSLOT):
            q_t = const.tile([128, 512], BF16, tag=f"qs{i}")
            nc.gpsimd.memset(q_t[H:128, :], 0.0)
            qslots.append(q_t)
            k_t = const.tile([128, 512], BF16, tag=f"ks{i}")
            nc.gpsimd.memset(k_t[H:128, :], 0.0)
            kslots.append(k_t)

        xt_pool = ctx.enter_context(tc.tile_pool(name="xt", bufs=6))
        p_pool = ctx.enter_context(tc.tile_pool(name="pp", bufs=3))
        o_pool = ctx.enter_context(tc.tile_pool(name="op", bufs=6))
        r_pool = ctx.enter_context(tc.tile_pool(name="rp", bufs=6))

        psqk_pool = ctx.enter_context(tc.tile_pool(name="psqk", bufs=2, space="PSUM"))
        psv_pool = ctx.enter_context(tc.tile_pool(name="psv", bufs=2, space="PSUM"))
        psc_pool = ctx.enter_context(tc.tile_pool(name="psc", bufs=2, space="PSUM"))
        pso_pool = ctx.enter_context(tc.tile_pool(name="pso", bufs=2, space="PSUM"))

        st = [dict() for _ in range(npair)]

        def s0_dma(pr):
            xt = xt_pool.tile([128, NCHUNK, 2 * T], BF16, tag="xt")
            nc.sync.dma_start(xt[:], xT[pr])
            st[pr]["xt"] = xt

        def s1_proj(pr):
            xt = st[pr]["xt"]
            # q^T|k^T for the pair: [128 qk-dims, 512 tokens]
            ps_qk = psqk_pool.tile([128, 512], F32, tag="psqk")
            for c in range(NCHUNK):
                nc.tensor.matmul(
                    ps_qk[:],
                    lhsT=wqk_sb[:, c, :],
                    rhs=xt[:, c, :],
                    start=(c == 0),
                    stop=(c == NCHUNK - 1),
                )
            # v [s, h] via x-stationary matmuls: group g = 2*b2 + s
            ps_v = psv_pool.tile([128, 512], F32, tag="psv")
            for g in range(4):
                b2, s = divmod(g, 2)
                t0 = b2 * T + s * 128
                for c in range(NCHUNK):
                    nc.tensor.matmul(
                        ps_v[:, g * H : (g + 1) * H],
                        lhsT=xt[:, c, t0 : t0 + 128],
                        rhs=wv_sb[:, c, :],
                        start=(c == 0),
                        stop=(c == NCHUNK - 1),
                    )
            st[pr]["ps_qk"] = ps_qk
            st[pr]["ps_v"] = ps_v

        def s2_copies(pr):
            ps_qk = st[pr]["ps_qk"]
            ps_v = st[pr]["ps_v"]
            q_sb = qslots[pr % NSLOT]
            nc.scalar.copy(q_sb[0:H, :], ps_qk[0:H, :])
            k_sb = kslots[pr % NSLOT]
            nc.vector.tensor_copy(k_sb[0:H, :], ps_qk[H:128, :])
            v_sb = vaug[pr % NSLOT]
            nc.vector.tensor_copy(
                v_sb[:, :, 0:H],
                ps_v[:, 0:256].rearrange("p (g h) -> p g h", g=4),
            )
            st[pr]["q"] = q_sb
            st[pr]["k"] = k_sb
            st[pr]["v"] = v_sb

        def s3_wei(pr):
            q_sb, k_sb = st[pr]["q"], st[pr]["k"]
            p_sb = p_pool.tile([128, 2, 384], BF16, tag="p")
            for b2 in range(2):
                toff = b2 * T
                # wei blocks [s, t]: A|B = (s0, t0:256) one N=256 matmul,
                # C = (s1, t1) one N=128 matmul. (t<128 never sees s1.)
                ps_c = psc_pool.tile([128, 512], F32, tag="psc")
                nc.tensor.matmul(
                    ps_c[:, 0:256],
                    lhsT=k_sb[:, toff : toff + 128],
                    rhs=q_sb[:, toff : toff + T],
                    start=True,
                    stop=True,
                )
                nc.tensor.matmul(
                    ps_c[:, 256:384],
                    lhsT=k_sb[:, toff + 128 : toff + 256],
                    rhs=q_sb[:, toff + 128 : toff + 256],
                    start=True,
                    stop=True,
                )
                # P = exp(wei * scale)
                nc.scalar.activation(
                    p_sb[:, b2, :], ps_c[:, 0:384],
                    mybir.ActivationFunctionType.Exp, scale=SCALE,
                )
            # causal mask on the 4 diagonal blocks (A at 0:128, C at
            # 256:384 per batch) in a single strided GpSimd op; out-MMs run
            # a tick later, so waiting on both exps costs nothing here.
            pv = p_sb[:].rearrange("p b (blk x) -> p b blk x", x=128)[:, :, 0:3:2, :]
            mv = mask_sb[:].rearrange("p (one blk x) -> p one blk x", one=1, x=128)
            pva, mva = broadcast_tensor_aps(pv, mv)
            nc.gpsimd.tensor_mul(pva, pva, mva)
            st[pr]["p"] = p_sb

        def s4_out(pr):
            v_sb = st[pr]["v"]
            p_sb = st[pr]["p"]
            ps_o = pso_pool.tile([128, 512], F32, tag="pso")
            for b2 in range(2):
                g0 = 2 * b2
                # out[t, 0:64] + denom[t] (col 64) via P-stationary matmuls
                nc.tensor.matmul(
                    ps_o[:, g0 * GW : g0 * GW + GW],
                    lhsT=p_sb[:, b2, 0:128],
                    rhs=v_sb[:, g0, :],
                    start=True,
                    stop=True,
                )
                nc.tensor.matmul(
                    ps_o[:, (g0 + 1) * GW : (g0 + 2) * GW],
                    lhsT=p_sb[:, b2, 128:256],
                    rhs=v_sb[:, g0, :],
                    start=True,
                    stop=False,
                )
                nc.tensor.matmul(
                    ps_o[:, (g0 + 1) * GW : (g0 + 2) * GW],
                    lhsT=p_sb[:, b2, 256:384],
                    rhs=v_sb[:, g0 + 1, :],
                    start=False,
                    stop=True,
                )
            st[pr]["ps_o"] = ps_o

        def s5_norm(pr):
            ps_o = st[pr]["ps_o"]
            # normalize: out[t, h] / denom[t] for all 4 groups at once
            og = ps_o[:, 0 : 4 * GW].rearrange("p (g c) -> p g c", g=4)
            rs = r_pool.tile([128, 4], F32, tag="rs")
            rsv = rs[:].rearrange("p (g c) -> p g c", c=1)
            nc.vector.reciprocal(rsv, og[:, :, H : H + 1])
            out_sb = o_pool.tile([128, 4, H], F32, tag="out")
            in0, in1 = broadcast_tensor_aps(og[:, :, 0:H], rsv)
            nc.vector.tensor_mul(out_sb[:], in0, in1)
            st[pr]["out"] = out_sb

        def s6_store(pr):
            # A tick after norm, so this never stalls the Sync FIFO and the
            # next tick's input DMA issues immediately behind it.
            nc.sync.dma_start(outH[pr], st[pr]["out"][:])
            st[pr].clear()

        stages = [s0_dma, s1_proj, s2_copies, s3_wei, s4_out, s5_norm, s6_store]
        for t in range(npair + len(stages) - 1):
            for off, stage in enumerate(stages):
                pr = t - off
                if 0 <= pr < npair:
                    stage(pr)

    nc.finalize()
    return nc


def _host_inputs(x, Wq, Wk, Wv):
    B_, T_, C_ = x.shape
    assert (B_, T_, C_) == (B, T, C), (B_, T_, C_)
    # xh[pr, p, c, 256*b2 + t] = x[2*pr + b2, t, 128*c + p]
    xh = np.ascontiguousarray(
        x.reshape(B // 2, 2, T, NCHUNK, 128).transpose(0, 4, 3, 1, 2)
        .reshape(B // 2, 128, NCHUNK, 2 * T)
        .astype(BF)
    )
    wqk_h = np.ascontiguousarray(
        np.concatenate([Wq, Wk], axis=1).reshape(NCHUNK, 128, 128).transpose(1, 0, 2)
        .astype(BF)
    )
    wv_h = np.ascontiguousarray(
        Wv.reshape(NCHUNK, 128, H).transpose(1, 0, 2).astype(BF)
    )
    triu = np.triu(np.ones((128, 128), dtype=BF))
    mask_h = np.ascontiguousarray(np.concatenate([triu, triu], axis=1))
    return xh, wqk_h, wv_h, mask_h


def _make_in_maps(xh, wqk_h, wv_h, mask_h):
    return [
        {
            "xT": xh[i * NPAIR : (i + 1) * NPAIR],
            "wqk": wqk_h,
            "wv": wv_h,
            "mask": mask_h,
        }
        for i in range(N_CORES)
    ]


def _assemble(results):
    # outH per core: [NPAIR, 128, 256]; groups g = 2*b2 + thalf
    outH = np.concatenate([results[i]["outH"] for i in range(N_CORES)], axis=0)
    out = (
        outH.reshape(B // 2, 128, 2, 2, H)
        .transpose(0, 2, 3, 1, 4)
        .reshape(B, T, H)
    )
    return np.ascontiguousarray(out)


def kernel(x, Wq, Wk, Wv):
    x = np.asarray(x, dtype=np.float32)
    Wq = np.asarray(Wq, dtype=np.float32)
    Wk = np.asarray(Wk, dtype=np.float32)
    Wv = np.asarray(Wv, dtype=np.float32)

    xh, wqk_h, wv_h, mask_h = _host_inputs(x, Wq, Wk, Wv)
    nc = build_nc(NPAIR)
    in_maps = _make_in_maps(xh, wqk_h, wv_h, mask_h)
    res = run_bass_kernel_spmd(nc, in_maps, list(range(N_CORES)))
    return _assemble(res.results)


# revision 40
# speedup vs baseline: 1.1437x; 1.0049x over previous
"""Single-head causal self-attention on 8 Trainium2 NeuronCores (Bass/Tile).

Problem: x [1024, 256, 384], Wq/Wk/Wv [384, 64] ->
  q,k,v = x@W;  wei = softmax(mask(q k^T / sqrt(384)));  out = wei @ v
Output: [1024, 256, 64] fp32.

Strategy (data-parallel over batch, 128 batches = 64 pairs per core):
  - Layout-by-stationarity: every tensor is produced in exactly the layout
    its consumer needs, so there are NO transposes anywhere.
      q^T,k^T [h, t]  = W-stationary matmul   (lhsT = [Wq|Wk] chunk)
      v       [s, h]  = x-stationary matmul   (lhsT = x^T chunk)
      P       [s, t]  = k-stationary matmul   (lhsT = k^T s-half) + exp
      out     [t, h]  = P-stationary matmul   (lhsT = P block, rhs = [v|1])
  - The ones column appended to v makes the out-MM emit the softmax
    denominator per OUTPUT PARTITION (= token), so normalization is one
    cheap reciprocal [128, 4, 1] + one broadcast tensor_mul per pair.
  - Causal structure at 128-block granularity: tokens t<128 never touch
    s-half1, so wei needs only 3 of 4 blocks (A=s0xt0 diag, B=s0xt1 full,
    C=s1xt1 diag); exp covers [128, 384]; mask [triu|triu] on A,C only.
  - bf16 operand pipeline (fp32 PSUM accumulation): 1 elem/cycle PE
    streaming (fp32r is ~2 cyc/elem), FWL fast weight loads, half HBM.
  - Software-pipelined emission: the pair loop is skewed into 6 stages
    (DMA / projections / PSUM->SBUF copies / wei+exp+mask / out-MMs /
    normalize+store) so each engine sees work from several pairs back to
    back. The projection stage alone is 15 dense matmuls, which keeps the
    PE HAM clock-gate warm (K=8/8).
"""

import os
from contextlib import ExitStack

import ml_dtypes
import numpy as np

import concourse.bass as bass
import concourse.bacc as bacc
import concourse.tile as tile
from concourse import mybir
from concourse.bass import broadcast_tensor_aps
from concourse.bass_utils import run_bass_kernel_spmd

N_CORES = 8
B = 1024
T = 256
C = 384
H = 64
BPC = B // N_CORES  # 128 batches per core
NPAIR = BPC // 2  # 64 pairs per core
NCHUNK = C // 128  # 3
SCALE = float(C) ** -0.5
GW = H + 2  # [v | 1 | pad] group width (even free dim, 8B-aligned groups)

F32 = mybir.dt.float32
BF16 = mybir.dt.bfloat16
BF = ml_dtypes.bfloat16


def build_nc(npair: int = NPAIR):
    nc = bacc.Bacc(
        "TRN2", target_bir_lowering=False, debug=False, num_devices=N_CORES
    )

    # xT[pr, p, c, 256*b2 + t] = x[2*pr + b2, t, 128*c + p]
    xT = nc.dram_tensor("xT", [npair, 128, NCHUNK, 2 * T], BF16, kind="ExternalInput").ap()
    wqk = nc.dram_tensor("wqk", [128, NCHUNK, 128], BF16, kind="ExternalInput").ap()
    wv = nc.dram_tensor("wv", [128, NCHUNK, H], BF16, kind="ExternalInput").ap()
    # [triu | triu] for the A and C (diagonal) block columns of P
    mask = nc.dram_tensor("mask", [128, 256], BF16, kind="ExternalInput").ap()
    # outH[pr, p, 64*g + h] = out[2*pr + g//2, 128*(g%2) + p, h]
    outH = nc.dram_tensor("outH", [npair, 128, 4 * H], F32, kind="ExternalOutput").ap()

    with ExitStack() as ctx:
        tc = ctx.enter_context(tile.TileContext(nc))

        const = ctx.enter_context(tc.tile_pool(name="const", bufs=1))
        wqk_sb = const.tile([128, NCHUNK, 128], BF16, tag="wqk")
        nc.sync.dma_start(wqk_sb[:], wqk)
        wv_sb = const.tile([128, NCHUNK, H], BF16, tag="wv")
        nc.sync.dma_start(wv_sb[:], wv)
        mask_sb = const.tile([128, 256], BF16, tag="mask")
        nc.sync.dma_start(mask_sb[:], mask)

        # Persistent [v | 1 | pad] rhs tiles: 4 groups (b0s0, b0s1, b1s0,
        # b1s1), each [128, 66] with ones at cols 64:66.
        NSLOT = 4
        vaug = []
        for i in range(NSLOT):
            v_t = const.tile([128, 4, GW], BF16, tag=f"vaug{i}")
            nc.gpsimd.memset(v_t[:, :, H : H + 2], 1.0)
            vaug.append(v_t)

        # q^T/k^T slots padded to 128 partitions with persistent zero rows:
        # the wei matmuls then contract over K=128, which qualifies their
        # weight loads for FWL (fast weight load needs 128 weight rows).
        qslots, kslots = [], []
        for i in range(NSLOT):
            q_t = const.tile([128, 512], BF16, tag=f"qs{i}")
            nc.gpsimd.memset(q_t[H:128, :], 0.0)
            qslots.append(q_t)
            k_t = const.tile([128, 512], BF16, tag=f"ks{i}")
            nc.gpsimd.memset(k_t[H:128, :], 0.0)
            kslots.append(k_t)

        xt_pool = ctx.enter_context(tc.tile_pool(name="xt", bufs=6))
        p_pool = ctx.enter_context(tc.tile_pool(name="pp", bufs=3))
        o_pool = ctx.enter_context(tc.tile_pool(name="op", bufs=6))
        r_pool = ctx.enter_context(tc.tile_pool(name="rp", bufs=6))

        psqk_pool = ctx.enter_context(tc.tile_pool(name="psqk", bufs=2, space="PSUM"))
        psv_pool = ctx.enter_context(tc.tile_pool(name="psv", bufs=2, space="PSUM"))
        psc_pool = ctx.enter_context(tc.tile_pool(name="psc", bufs=2, space="PSUM"))
        pso_pool = ctx.enter_context(tc.tile_pool(name="pso", bufs=2, space="PSUM"))

        st = [dict() for _ in range(npair)]

        def s0_dma(pr):
            xt = xt_pool.tile([128, NCHUNK, 2 * T], BF16, tag="xt")
            nc.sync.dma_start(xt[:], xT[pr])
            st[pr]["xt"] = xt

        def s1_proj(pr):
            xt = st[pr]["xt"]
            # q^T|k^T for the pair: [128 qk-dims, 512 tokens]
            ps_qk = psqk_pool.tile([128, 512], F32, tag="psqk")
            for c in range(NCHUNK):
                nc.tensor.matmul(
                    ps_qk[:],
                    lhsT=wqk_sb[:, c, :],
                    rhs=xt[:, c, :],
                    start=(c == 0),
                    stop=(c == NCHUNK - 1),
                )
            # v [s, h] via x-stationary matmuls: group g = 2*b2 + s
            ps_v = psv_pool.tile([128, 512], F32, tag="psv")
            for g in range(4):
                b2, s = divmod(g, 2)
                t0 = b2 * T + s * 128
                for c in range(NCHUNK):
                    nc.tensor.matmul(
                        ps_v[:, g * H : (g + 1) * H],
                        lhsT=xt[:, c, t0 : t0 + 128],
                        rhs=wv_sb[:, c, :],
                        start=(c == 0),
                        stop=(c == NCHUNK - 1),
                    )
            st[pr]["ps_qk"] = ps_qk
            st[pr]["ps_v"] = ps_v

        def s2_copies(pr):
            ps_qk = st[pr]["ps_qk"]
            ps_v = st[pr]["ps_v"]
            q_sb = qslots[pr % NSLOT]
            nc.scalar.copy(q_sb[0:H, :], ps_qk[0:H, :])
            k_sb = kslots[pr % NSLOT]
            nc.vector.tensor_copy(k_sb[0:H, :], ps_qk[H:128, :])
            v_sb = vaug[pr % NSLOT]
            nc.vector.tensor_copy(
                v_sb[:, :, 0:H],
                ps_v[:, 0:256].rearrange("p (g h) -> p g h", g=4),
            )
            st[pr]["q"] = q_sb
            st[pr]["k"] = k_sb
            st[pr]["v"] = v_sb

        def s3_wei(pr):
            q_sb, k_sb = st[pr]["q"], st[pr]["k"]
            p_sb = p_pool.tile([128, 2, 384], BF16, tag="p")
            for b2 in range(2):
                toff = b2 * T
                # wei blocks [s, t]: A|B = (s0, t0:256) one N=256 matmul,
                # C = (s1, t1) one N=128 matmul. (t<128 never sees s1.)
                ps_c = psc_pool.tile([128, 512], F32, tag="psc")
                nc.tensor.matmul(
                    ps_c[:, 0:256],
                    lhsT=k_sb[:, toff : toff + 128],
                    rhs=q_sb[:, toff : toff + T],
                    start=True,
                    stop=True,
                )
                nc.tensor.matmul(
                    ps_c[:, 256:384],
                    lhsT=k_sb[:, toff + 128 : toff + 256],
                    rhs=q_sb[:, toff + 128 : toff + 256],
                    start=True,
                    stop=True,
                )
                # P = exp(wei * scale)
                nc.scalar.activation(
                    p_sb[:, b2, :], ps_c[:, 0:384],
                    mybir.ActivationFunctionType.Exp, scale=SCALE,
                )
            # causal mask on the 4 diagonal blocks (A at 0:128, C at
            # 256:384 per batch) in a single strided GpSimd op; out-MMs run
            # a tick later, so waiting on both exps costs nothing here.
            pv = p_sb[:].rearrange("p b (blk x) -> p b blk x", x=128)[:, :, 0:3:2, :]
            mv = mask_sb[:].rearrange("p (one blk x) -> p one blk x", one=1, x=128)
            pva, mva = broadcast_tensor_aps(pv, mv)
            nc.gpsimd.tensor_mul(pva, pva, mva)
            st[pr]["p"] = p_sb

        def s4_out(pr):
            v_sb = st[pr]["v"]
            p_sb = st[pr]["p"]
            ps_o = pso_pool.tile([128, 512], F32, tag="pso")
            for b2 in range(2):
                g0 = 2 * b2
                # out[t, 0:64] + denom[t] (col 64) via P-stationary matmuls
                nc.tensor.matmul(
                    ps_o[:, g0 * GW : g0 * GW + GW],
                    lhsT=p_sb[:, b2, 0:128],
                    rhs=v_sb[:, g0, :],
                    start=True,
                    stop=True,
                )
                nc.tensor.matmul(
                    ps_o[:, (g0 + 1) * GW : (g0 + 2) * GW],
                    lhsT=p_sb[:, b2, 128:256],
                    rhs=v_sb[:, g0, :],
                    start=True,
                    stop=False,
                )
                nc.tensor.matmul(
                    ps_o[:, (g0 + 1) * GW : (g0 + 2) * GW],
                    lhsT=p_sb[:, b2, 256:384],
                    rhs=v_sb[:, g0 + 1, :],
                    start=False,
                    stop=True,
                )
            st[pr]["ps_o"] = ps_o

        def s5_norm(pr):
            ps_o = st[pr]["ps_o"]
            # normalize: out[t, h] / denom[t] for all 4 groups at once
            og = ps_o[:, 0 : 4 * GW].rearrange("p (g c) -> p g c", g=4)
            rs = r_pool.tile([128, 4], F32, tag="rs")
            rsv = rs[:].rearrange("p (g c) -> p g c", c=1)
            nc.vector.reciprocal(rsv, og[:, :, H : H + 1])
            out_sb = o_pool.tile([128, 4, H], F32, tag="out")
            in0, in1 = broadcast_tensor_aps(og[:, :, 0:H], rsv)
            nc.vector.tensor_mul(out_sb[:], in0, in1)
            st[pr]["out"] = out_sb

        def s6_store(pr):
            # A tick after norm, so this never stalls the Sync FIFO and the
            # next tick's input DMA issues immediately behind it.
            nc.sync.dma_start(outH[pr], st[pr]["out"][:])
            st[pr].clear()

        # s4 sits at offset 5 (not 4): the GpSimd mask of s3 runs ~1.15us
        # late in its tick, so the out-MMs get a full extra tick of slack
        # and never stall on it.
        stages = [
            (0, s0_dma),
            (1, s1_proj),
            (2, s2_copies),
            (3, s3_wei),
            (5, s4_out),
            (6, s5_norm),
            (7, s6_store),
        ]
        for t in range(npair + 7):
            for off, stage in stages:
                pr = t - off
                if 0 <= pr < npair:
                    stage(pr)

    nc.finalize()
    return nc


def _host_inputs(x, Wq, Wk, Wv):
    B_, T_, C_ = x.shape
    assert (B_, T_, C_) == (B, T, C), (B_, T_, C_)
    # xh[pr, p, c, 256*b2 + t] = x[2*pr + b2, t, 128*c + p]
    xh = np.ascontiguousarray(
        x.reshape(B // 2, 2, T, NCHUNK, 128).transpose(0, 4, 3, 1, 2)
        .reshape(B // 2, 128, NCHUNK, 2 * T)
        .astype(BF)
    )
    wqk_h = np.ascontiguousarray(
        np.concatenate([Wq, Wk], axis=1).reshape(NCHUNK, 128, 128).transpose(1, 0, 2)
        .astype(BF)
    )
    wv_h = np.ascontiguousarray(
        Wv.reshape(NCHUNK, 128, H).transpose(1, 0, 2).astype(BF)
    )
    triu = np.triu(np.ones((128, 128), dtype=BF))
    mask_h = np.ascontiguousarray(np.concatenate([triu, triu], axis=1))
    return xh, wqk_h, wv_h, mask_h


def _make_in_maps(xh, wqk_h, wv_h, mask_h):
    return [
        {
            "xT": xh[i * NPAIR : (i + 1) * NPAIR],
            "wqk": wqk_h,
            "wv": wv_h,
            "mask": mask_h,
        }
        for i in range(N_CORES)
    ]


def _assemble(results):
    # outH per core: [NPAIR, 128, 256]; groups g = 2*b2 + thalf
    outH = np.concatenate([results[i]["outH"] for i in range(N_CORES)], axis=0)
    out = (
        outH.reshape(B // 2, 128, 2, 2, H)
        .transpose(0, 2, 3, 1, 4)
        .reshape(B, T, H)
    )
    return np.ascontiguousarray(out)


def kernel(x, Wq, Wk, Wv):
    x = np.asarray(x, dtype=np.float32)
    Wq = np.asarray(Wq, dtype=np.float32)
    Wk = np.asarray(Wk, dtype=np.float32)
    Wv = np.asarray(Wv, dtype=np.float32)

    xh, wqk_h, wv_h, mask_h = _host_inputs(x, Wq, Wk, Wv)
    nc = build_nc(NPAIR)
    in_maps = _make_in_maps(xh, wqk_h, wv_h, mask_h)
    res = run_bass_kernel_spmd(nc, in_maps, list(range(N_CORES)))
    return _assemble(res.results)
